# revision 4
# baseline (speedup 1.0000x reference)
"""Trainium2 Bass kernel for loss = sum((X[:,None]*A - I)**2), N=8192.

Algebraic decomposition (avoids materializing the residual):
    loss = sum_ij (x_i*a_ij)^2  -  2*sum_i x_i*a_ii  +  N
         = sum_i x_i^2 * r_i    -  2*sum_i x_i*d_i   +  N
where r_i = sum_j a_ij^2 (row sums of squares) and d_i = a_ii.

The device computes ONLY r_i (row sums of squares of A) — the O(N^2)
work — streaming each core's 32 MB shard from HBM exactly once.  The
O(N) terms (x_i^2 fold, diagonal correction, +N) are folded on the host
in float64, which also improves the final-sum numerics.

Sharding: A row-wise across 8 cores (1024 rows each).  Per core the
shard is processed in [128, CH] column-chunks of the 8 row-groups;
ScalarE's fused activation(Square, accum_out) produces each chunk's
per-row partial sums in one pass.  Chunks are sized so the DMA stream
(~358 GB/s per-core HBM limit) stays saturated while the tail — the
last chunk's activation after the final DMA lands — is short.  The
[128, NCH] partial-sum tile is DMA'd out; the host maps (chunk, p) back
to rows and reduces in float64.
"""

import numpy as np

import concourse.bacc as bacc
import concourse.mybir as mybir
from concourse.tile import TileContext
from concourse.bass_utils import run_bass_kernel_spmd

N = 8192
NCORES = 8
ROWS = N // NCORES  # 1024 rows per core
P = 128  # SBUF partitions
GROUPS = ROWS // P  # 8 row-groups of 128 rows per core
CH = 2048  # columns per chunk: [128, 2048] f32 = 1 MiB per DMA
CPG = N // CH  # chunks per row-group
NCH = GROUPS * CPG  # chunks (= racc columns) per core

_DT = mybir.dt.float32


def chunk_plan(ch=CH, taper=()):
    """List of (group, col0, cols) chunks covering [ROWS, N] exactly once.

    Groups 0..GROUPS-2 are split uniformly into `ch`-column chunks; the
    LAST group uses `taper` (col widths summing to N) if given, so the
    final activations — which run after the last DMA lands — are short.
    """
    plan = []
    for t in range(GROUPS - 1):
        for c0 in range(0, N, ch):
            plan.append((t, c0, ch))
    t = GROUPS - 1
    if taper:
        assert sum(taper) == N
        c0 = 0
        for cols in taper:
            plan.append((t, c0, cols))
            c0 += cols
    else:
        for c0 in range(0, N, ch):
            plan.append((t, c0, ch))
    return plan


def build_nc(reps=1, ch=CH, bufs=4, taper=()):
    """reps>1 repeats the whole per-core computation in one NEFF; used by
    the timing harness to measure per-iteration device time by slope."""
    nc = bacc.Bacc("TRN2", target_bir_lowering=False)

    plan = chunk_plan(ch, taper)
    nch = len(plan)

    a_shard = nc.dram_tensor("a_shard", [ROWS, N], _DT, kind="ExternalInput")
    out = nc.dram_tensor("out", [P, nch * reps], _DT, kind="ExternalOutput")

    a_tiles = a_shard.rearrange("(t p) n -> t p n", p=P)

    with TileContext(nc) as tc:
        with (
            tc.tile_pool(name="a", bufs=bufs) as apool,
            tc.tile_pool(name="small", bufs=1) as small,
        ):
            racc = small.tile([P, nch], _DT, tag="racc")
            # Throwaway full-size output for the fused square+reduce:
            # stride-0 broadcast of a [P,1] tile, so no [P,ch] scratch is
            # needed.
            dummy = small.tile([P, 1], _DT, tag="dummy")

            for _rep in range(reps):
                for c, (t, c0, cols) in enumerate(plan):
                    at = apool.tile([P, cols], _DT, tag="a", name=f"at{c}")
                    nc.sync.dma_start(
                        out=at[:], in_=a_tiles[t][:, c0 : c0 + cols]
                    )
                    nc.scalar.activation(
                        out=dummy.broadcast_to(at.shape),
                        in_=at[:],
                        func=mybir.ActivationFunctionType.Square,
                        accum_out=racc[:, c : c + 1],
                    )

                # Ship the [128, nch] per-(row, chunk) partials; the host
                # folds x^2, the diagonal term, and the final sum in float64.
                nc.sync.dma_start(
                    out=out[:, _rep * nch : (_rep + 1) * nch], in_=racc[:]
                )

    nc.compile()
    return nc


_nc_cache = {}

# The deployed configuration (see module docstring).
CONFIG = {"ch": CH, "bufs": 4, "taper": ()}


def _get_nc(reps=1, **over):
    cfg = {**CONFIG, **over}
    key = (reps, cfg["ch"], cfg["bufs"], tuple(cfg["taper"]))
    if key not in _nc_cache:
        _nc_cache[key] = build_nc(reps, **cfg)
    return _nc_cache[key]


def _shard_inputs(A):
    A = np.ascontiguousarray(np.asarray(A, dtype=np.float32))
    return [{"a_shard": A[c * ROWS : (c + 1) * ROWS]} for c in range(NCORES)]


def _fold_host(X, A, raccs, plan):
    """loss = sum_i x_i^2 r_i - 2 sum_i x_i a_ii + N, in float64."""
    x64 = np.asarray(X, dtype=np.float64)
    d64 = np.ascontiguousarray(np.asarray(A).diagonal()).astype(np.float64)
    total = 0.0
    for c, racc in enumerate(raccs):
        r64 = racc.astype(np.float64)  # [P, nch]
        # racc column j holds sum over cols [c0, c0+cols) of row
        # (c*ROWS + t*P + p); accumulate per row-group.
        rg = np.zeros((P, GROUPS))
        for j, (t, _c0, _cols) in enumerate(plan):
            rg[:, t] += r64[:, j]
        xg = x64[c * ROWS : (c + 1) * ROWS].reshape(GROUPS, P).T  # [P, GROUPS]
        total += float((xg * xg * rg).sum())
    total += float(N) - 2.0 * float(np.dot(x64, d64))
    return np.float32(total)


def _run(inputs, trace=False):
    nc = _get_nc()
    plan = chunk_plan(CONFIG["ch"], CONFIG["taper"])
    in_maps = _shard_inputs(inputs["A"])
    res = run_bass_kernel_spmd(
        nc, in_maps, core_ids=list(range(NCORES)), trace=trace
    )
    raccs = [r["out"][:, : len(plan)] for r in res.results]
    total = _fold_host(inputs["X"], inputs["A"], raccs, plan)
    return np.array(total, dtype=np.float32), res


def kernel(**inputs):
    out, _ = _run(inputs, trace=False)
    return out


# revision 12
# speedup vs baseline: 232.9795x; 232.9795x over previous
"""Trainium2 Bass kernel for loss = sum((X[:,None]*A - I)**2), N=8192.

Algebraic decomposition (avoids materializing the residual):
    loss = sum_ij (x_i*a_ij)^2  -  2*sum_i x_i*a_ii  +  N
         = sum_i x_i^2 * r_i    -  2*sum_i x_i*d_i   +  N
where r_i = sum_j a_ij^2 (row sums of squares) and d_i = a_ii.

The device computes ONLY r_i (row sums of squares of A) — the O(N^2)
work — streaming each core's 32 MB shard from HBM exactly once.  The
O(N) terms (x_i^2 fold, diagonal correction, +N) are folded on the host
in float64, which also improves the final-sum numerics.

Sharding: A row-wise across 8 cores (1024 rows each).  Per core the
shard is processed in [128, cols] column-chunks of the 8 row-groups;
ScalarE's fused activation(Square, accum_out) produces each chunk's
per-row partial sums in one pass.  The kernel is DMA-bandwidth-bound
(~358 GB/s per-core HBM share -> 93 us for the 32 MiB shard; measured
~103 us/rep steady-state on HW, cost-model single-shot span 100.3 us),
so the tuning is all about the fixed head/tail around the DMA stream:

- 1 MiB body chunks keep each DMA well above the ~1.3 us per-DMA
  pipeline floor (smaller chunks stretch the stream; larger ones only
  lengthen the tail).
- The last row-group is tapered into 0.5 MiB chunks and the final two
  chunks are squared concurrently on ScalarE and DVE, so the
  post-stream activation tail is ~1 us instead of 7 us.
- x^2, the diagonal term, and +N are folded on the HOST in float64
  (they are O(N)); the device ships only the [128, nch] partial-sum
  tile.  No x/diag inputs -> the first A-tile DMA issues ~1.3 us
  earlier.

vs. the previous version (x/diag on device, 4 MiB chunks, DVE
epilogue): 108.3 us -> 100.3 us cost-model span.
"""

import numpy as np

import concourse.bacc as bacc
import concourse.mybir as mybir
from concourse.tile import TileContext
from concourse.bass_utils import run_bass_kernel_spmd

N = 8192
NCORES = 8
ROWS = N // NCORES  # 1024 rows per core
P = 128  # SBUF partitions
GROUPS = ROWS // P  # 8 row-groups of 128 rows per core
CH = 2048  # columns per chunk: [128, 2048] f32 = 1 MiB per DMA
CPG = N // CH  # chunks per row-group
NCH = GROUPS * CPG  # chunks (= racc columns) per core

_DT = mybir.dt.float32


def chunk_plan(ch=CH, taper=()):
    """List of (group, col0, cols) chunks covering [ROWS, N] exactly once.

    Groups 0..GROUPS-2 are split uniformly into `ch`-column chunks; the
    LAST group uses `taper` (col widths summing to N) if given, so the
    final activations — which run after the last DMA lands — are short.
    """
    plan = []
    for t in range(GROUPS - 1):
        for c0 in range(0, N, ch):
            plan.append((t, c0, ch))
    t = GROUPS - 1
    if taper:
        assert sum(taper) == N
        c0 = 0
        for cols in taper:
            plan.append((t, c0, cols))
            c0 += cols
    else:
        for c0 in range(0, N, ch):
            plan.append((t, c0, ch))
    return plan


def build_nc(reps=1, ch=CH, bufs=4, taper=(), engines="s", out_engine="sync"):
    """reps>1 repeats the whole per-core computation in one NEFF; used by
    the timing harness to measure per-iteration device time by slope.

    engines: which engines square+reduce the chunks, round-robin by
    pattern character — 's' ScalarE activation(Square), 'v' DVE
    scalar_tensor_tensor(mult), 'p' GpSimd scalar_tensor_tensor(mult).
    E.g. "svv" sends 1/3 of chunks to ScalarE, 2/3 to DVE.  A pattern
    exactly as long as the chunk plan assigns engines per-chunk (e.g.
    ...ending in "sv" runs the final two chunks concurrently on ScalarE
    and DVE, halving the post-DMA activation tail).
    """
    nc = bacc.Bacc("TRN2", target_bir_lowering=False)

    plan = chunk_plan(ch, taper)
    nch = len(plan)

    a_shard = nc.dram_tensor("a_shard", [ROWS, N], _DT, kind="ExternalInput")
    out = nc.dram_tensor("out", [P, nch * reps], _DT, kind="ExternalOutput")

    a_tiles = a_shard.rearrange("(t p) n -> t p n", p=P)

    with TileContext(nc) as tc:
        with (
            tc.tile_pool(name="a", bufs=bufs) as apool,
            tc.tile_pool(name="small", bufs=1) as small,
        ):
            racc = small.tile([P, nch], _DT, tag="racc")
            # Throwaway full-size outputs for the fused square+reduce:
            # stride-0 broadcast of a [P,1] tile, so no [P,ch] scratch is
            # needed.  One per engine so engines don't alias an output tile
            # (which would serialize them in the dependency tracker).
            dummies = {
                e: small.tile([P, 1], _DT, tag=f"dummy_{e}", name=f"dummy_{e}")
                for e in set(engines)
            }

            for _rep in range(reps):
                for c, (t, c0, cols) in enumerate(plan):
                    at = apool.tile([P, cols], _DT, tag="a", name=f"at{c}")
                    nc.sync.dma_start(
                        out=at[:], in_=a_tiles[t][:, c0 : c0 + cols]
                    )
                    e = engines[c % len(engines)]
                    acc = racc[:, c : c + 1]
                    dummy_bc = dummies[e].broadcast_to(at.shape)
                    if e == "s":
                        nc.scalar.activation(
                            out=dummy_bc,
                            in_=at[:],
                            func=mybir.ActivationFunctionType.Square,
                            accum_out=acc,
                        )
                    else:
                        eng = nc.vector if e == "v" else nc.gpsimd
                        eng.scalar_tensor_tensor(
                            out=dummy_bc,
                            in0=at[:],
                            scalar=1.0,
                            in1=at[:],
                            op0=mybir.AluOpType.mult,
                            op1=mybir.AluOpType.mult,
                            accum_out=acc,
                        )

                # Ship the [128, nch] per-(row, chunk) partials; the host
                # folds x^2, the diagonal term, and the final sum in float64.
                # Issued from the ACT sequencer (also HWDGE): when the final
                # chunk's square ran on ScalarE, the issue queues right
                # behind it with no cross-engine semaphore hop.
                out_eng = nc.scalar if out_engine == "scalar" else nc.sync
                out_eng.dma_start(
                    out=out[:, _rep * nch : (_rep + 1) * nch], in_=racc[:]
                )

    nc.compile()
    return nc


_nc_cache = {}

# The deployed configuration (see module docstring): 1 MiB body chunks,
# the last row-group split into 0.5 MiB chunks (short tail, DMA stream
# still packed), final two chunks squared concurrently on ScalarE + DVE.
_NCH = (GROUPS - 1) * (N // CH) + 8
CONFIG = {
    "ch": CH,
    "bufs": 4,
    "taper": (1024,) * 8,
    "engines": "s" * (_NCH - 2) + "sv",
}


def _get_nc(reps=1, **over):
    cfg = {**CONFIG, **over}
    key = (reps, *sorted(cfg.items()))
    if key not in _nc_cache:
        _nc_cache[key] = build_nc(reps, **cfg)
    return _nc_cache[key]


def _shard_inputs(A):
    A = np.ascontiguousarray(np.asarray(A, dtype=np.float32))
    return [{"a_shard": A[c * ROWS : (c + 1) * ROWS]} for c in range(NCORES)]


def _fold_host(X, A, raccs, plan):
    """loss = sum_i x_i^2 r_i - 2 sum_i x_i a_ii + N, in float64."""
    x64 = np.asarray(X, dtype=np.float64)
    d64 = np.ascontiguousarray(np.asarray(A).diagonal()).astype(np.float64)
    total = 0.0
    for c, racc in enumerate(raccs):
        r64 = racc.astype(np.float64)  # [P, nch]
        # racc column j holds sum over cols [c0, c0+cols) of row
        # (c*ROWS + t*P + p); accumulate per row-group.
        rg = np.zeros((P, GROUPS))
        for j, (t, _c0, _cols) in enumerate(plan):
            rg[:, t] += r64[:, j]
        xg = x64[c * ROWS : (c + 1) * ROWS].reshape(GROUPS, P).T  # [P, GROUPS]
        total += float((xg * xg * rg).sum())
    total += float(N) - 2.0 * float(np.dot(x64, d64))
    return np.float32(total)


def _run(inputs, trace=False):
    nc = _get_nc()
    plan = chunk_plan(CONFIG["ch"], CONFIG["taper"])
    in_maps = _shard_inputs(inputs["A"])
    res = run_bass_kernel_spmd(
        nc, in_maps, core_ids=list(range(NCORES)), trace=trace
    )
    raccs = [r["out"][:, : len(plan)] for r in res.results]
    total = _fold_host(inputs["X"], inputs["A"], raccs, plan)
    return np.array(total, dtype=np.float32), res


def kernel(**inputs):
    out, _ = _run(inputs, trace=False)
    return out
